# revision 7
# baseline (speedup 1.0000x reference)
"""Trainium2 Bass kernel for nn_CalibratedNorm.

The reference module collapses algebraically to a per-(sample, channel)
affine:

    out[b,c,h,w] = x[b,c,h,w] * A[b,c] + S[b,c]

where, with gs/gsh the folded global-BN scale/shift and ms/msh the folded
mean-of-group-BNs scale/shift (all tiny [C] host math):

    alpha[b] = sigmoid( sum_c (alpha_w[c]/HW) * sum_hw x[b,c,:,:] + alpha_b )
    A[b,c]   = gs[c]  + alpha[b] * (ms[c]  - gs[c])
    S[b,c]   = gsh[c] + alpha[b] * (msh[c] - gsh[c])

Strategy: data-parallel over batch, 4 samples per core on 8 cores. The
kernel is DMA-ring bound (the ring bills SBUF-side bytes; ~425 GB/s
needs >= 12KB contiguous per partition row), so:

  * x travels the wire as fp16 both ways (host casts): 6.4 MB in +
    6.4 MB out per core vs 25.7 MB round trip in fp32; ~6e-4 relative
    error on this N(0,1) input, far inside the 2e-2 gate.
  * The host packs TWO samples per partition row
    [b0h0 | b0h1 | b1h0 | b1h1] so every DMA moves 25088B rows: two
    1.6MB... rather two 3.2MB loads and two 3.2MB stores run at peak
    packet efficiency (12544B fp16 rows measured ~360-390 GB/s; 25088B
    ~425 GB/s).
  * The alpha dot runs on the otherwise-idle PE as 14 accumulating
    matmuls per sample (lhsT = 2^16-prescaled weight column, rhs =
    448-pixel chunks of x), then a 448-elem DVE reduce + ACT sigmoid +
    PE ones-broadcast. DVE's 1x-rate reduce_sum over full samples
    (3.4us per half!) was the original critical path - the PE dot
    removes it.
  * All bulk affines run on DVE (fp16 tensor_scalar at 4x, ~0.8us per
    half-sample), so a pair's store can chase its second sample's
    affine ~1.6us after alpha.
"""

import sys

import numpy as np

for _p in ("/opt/trn_rl_repo",):
    if _p not in sys.path:
        sys.path.insert(0, _p)

import concourse.bacc as bacc
import concourse.bass as bass
import concourse.tile as tile
from concourse import mybir
from concourse.bass_utils import run_bass_kernel_spmd
from concourse.tile import add_dep_helper

EPS = 1e-5
B, C, H, W, G = 32, 256, 56, 56, 32
HW = H * W  # 3136
NCORES = 8
BPC = B // NCORES  # samples per core: 4
NPAIR = BPC // 2  # sample pairs per core: 2
HALVES = C // 128  # channel partition-tiles per sample: 2
ROWS = NPAIR * 128  # 256 rows of the per-core [ROWS, 2*HALVES*HW] x shard
NCHUNK = 7  # alpha-dot rhs chunks per half
CHUNK = HW // NCHUNK  # 448 pixels per chunk (PSUM-bank sized)
WSCALE = 65536.0  # alpha-weight prescale to keep fp16 normal
F32 = mybir.dt.float32
F16 = mybir.dt.float16


def build_module() -> bass.Bass:
    # Bacc (not raw Bass): its compile() pass splits multi-sem waits into
    # EventSemaphore instructions — TRN2 allows at most 1 wait per
    # compute instruction and walrus codegen hard-errors otherwise.
    nc = bacc.Bacc("TRN2")

    x_in = nc.dram_tensor("x", [ROWS, 2 * HALVES * HW], F16, kind="ExternalInput")
    wq_in = nc.dram_tensor("wq", [128, HALVES], F16, kind="ExternalInput")
    tab_in = nc.dram_tensor("tab", [128, 4, HALVES], F32, kind="ExternalInput")
    ab_in = nc.dram_tensor("ab", [1, 1], F32, kind="ExternalInput")
    ones_in = nc.dram_tensor("ones", [1, 128], F32, kind="ExternalInput")
    y_out = nc.dram_tensor("out", [ROWS, 2 * HALVES * HW], F16, kind="ExternalOutput")

    with tile.TileContext(nc) as tc:
        with (
            tc.tile_pool(name="xp", bufs=NPAIR) as xp,
            tc.tile_pool(name="cs", bufs=1) as cs,
            tc.tile_pool(name="wk", bufs=2) as wk,
            tc.tile_pool(name="ps", bufs=BPC, space="PSUM") as ps,
            tc.tile_pool(name="pb", bufs=2, space="PSUM") as pb,
        ):
            # Tiny param tables on the SWDGE queue so they never wait
            # behind the bulk x loads on the HWDGE ring.
            wq = cs.tile([128, HALVES], F16)
            nc.gpsimd.dma_start(out=wq, in_=wq_in[:, :])
            tab = cs.tile([128, 4, HALVES], F32)
            nc.gpsimd.dma_start(out=tab, in_=tab_in[:, :, :])
            ab = cs.tile([1, 1], F32)
            nc.gpsimd.dma_start(out=ab, in_=ab_in[:, :])
            ones_row = cs.tile([1, 128], F32)
            nc.gpsimd.dma_start(out=ones_row, in_=ones_in[:, :])

            # row r = j*128 + p holds samples 2j and 2j+1, partition p:
            # [b=2j: h0 pix, h1 pix | b=2j+1: h0 pix, h1 pix].
            # channel c = h*128 + p.
            xv = x_in[:, :].rearrange(
                "(j p) (i h w) -> j p i h w", p=128, i=2, h=HALVES
            )
            yv = y_out[:, :].rearrange(
                "(j p) (i h w) -> j p i h w", p=128, i=2, h=HALVES
            )

            loads = []
            stores = []
            for j in range(NPAIR):
                xt = xp.tile([128, 2, HALVES, HW], F16, name=f"xt{j}", tag="xt")
                # One 3.2MB DMA per pair: 25088B per partition row.
                loads.append(nc.sync.dma_start(out=xt[:, :, :, :], in_=xv[j]))

                for i in range(2):
                    b = 2 * j + i
                    # alpha dot on PE: zp[0,n] accumulates
                    #   sum_h sum_k sum_p wq[p,h] * x[p,i,h,448k+n]
                    zp = ps.tile([1, CHUNK], F32, name=f"zp{b}", tag="zp")
                    for h in range(HALVES):
                        for k in range(NCHUNK):
                            nc.tensor.matmul(
                                zp[:, :],
                                lhsT=wq[:, h : h + 1],
                                rhs=xt[:, i, h, k * CHUNK : (k + 1) * CHUNK],
                                start=(h == 0 and k == 0),
                                stop=(h == HALVES - 1 and k == NCHUNK - 1),
                            )
                    # z = sum_n zp[0,n]; alpha = sigmoid(z/WSCALE + ab)
                    zs = wk.tile([1, 1], F32, name=f"zs{b}", tag="zs")
                    nc.vector.reduce_sum(
                        out=zs, in_=zp[:, :], axis=mybir.AxisListType.X
                    )
                    al = wk.tile([1, 1], F32, name=f"al{b}", tag="al")
                    nc.scalar.activation(
                        out=al, in_=zs[:, :],
                        func=mybir.ActivationFunctionType.Sigmoid,
                        bias=ab[0:1, 0:1], scale=float(1.0 / WSCALE),
                    )
                    bc = pb.tile([128, 1], F32, name=f"bc{b}", tag="bc")
                    nc.tensor.matmul(
                        bc[:, :], lhsT=ones_row[:, :], rhs=al[:, :],
                        start=True, stop=True,
                    )
                    ac = wk.tile([128, 1], F32, name=f"ac{b}", tag="ac")
                    nc.vector.tensor_copy(out=ac, in_=bc[:, :])

                    # A = gs + alpha*dms ; S = gsh + alpha*dmsh   [128, 2]
                    A = wk.tile([128, HALVES], F32, name=f"A{b}", tag="A")
                    Sh = wk.tile([128, HALVES], F32, name=f"S{b}", tag="S")
                    nc.vector.tensor_scalar_mul(out=A, in0=tab[:, 1, :], scalar1=ac)
                    nc.vector.tensor_add(out=A, in0=A[:, :], in1=tab[:, 0, :])
                    nc.vector.tensor_scalar_mul(out=Sh, in0=tab[:, 3, :], scalar1=ac)
                    nc.vector.tensor_add(out=Sh, in0=Sh[:, :], in1=tab[:, 2, :])

                    # Fused affine on DVE (fp16 tensor_scalar at 4x).
                    for h in range(HALVES):
                        nc.vector.tensor_scalar(
                            out=xt[:, i, h, :], in0=xt[:, i, h, :],
                            scalar1=A[:, h : h + 1], scalar2=Sh[:, h : h + 1],
                            op0=mybir.AluOpType.mult, op1=mybir.AluOpType.add,
                        )
                stores.append(nc.sync.dma_start(out=yv[j], in_=xt[:, :, :, :]))

            # Keep every load ahead of every store in the HWDGE ring:
            # ordering-only edges (no sems) from each store to the last
            # load.
            for st in stores:
                add_dep_helper(
                    st.ins, loads[-1].ins, sync=False,
                    reason="loads drain before stores on SP ring",
                )

    nc.compile()
    return nc


_NC_CACHE: list = []


def _get_module() -> bass.Bass:
    if not _NC_CACHE:
        _NC_CACHE.append(build_module())
    return _NC_CACHE[0]


def _prep_in_maps(inputs: dict) -> list[dict]:
    x = np.asarray(inputs["x"], dtype=np.float32)
    alpha_w = np.asarray(inputs["alpha_w"], dtype=np.float32)
    alpha_b = np.asarray(inputs["alpha_b"], dtype=np.float32)
    g_w = np.asarray(inputs["g_w"], dtype=np.float32)
    g_b = np.asarray(inputs["g_b"], dtype=np.float32)
    g_rm = np.asarray(inputs["g_rm"], dtype=np.float32)
    g_rv = np.asarray(inputs["g_rv"], dtype=np.float32)
    grp_w = np.asarray(inputs["grp_w"], dtype=np.float32)
    grp_b = np.asarray(inputs["grp_b"], dtype=np.float32)
    grp_rm = np.asarray(inputs["grp_rm"], dtype=np.float32)
    grp_rv = np.asarray(inputs["grp_rv"], dtype=np.float32)

    gs = g_w / np.sqrt(g_rv + EPS)
    gsh = g_b - g_rm * gs
    sg = grp_w / np.sqrt(grp_rv + EPS)  # [G, C]
    ms = sg.mean(axis=0)
    msh = (grp_b - grp_rm * sg).mean(axis=0)
    dms = ms - gs
    dmsh = msh - gsh

    ch = (np.arange(HALVES)[None, :] * 128 + np.arange(128)[:, None])  # [128, HALVES]
    tab = np.empty((128, 4, HALVES), dtype=np.float32)
    tab[:, 0, :] = gs[ch]
    tab[:, 1, :] = dms[ch]
    tab[:, 2, :] = gsh[ch]
    tab[:, 3, :] = dmsh[ch]

    wq = ((alpha_w * (WSCALE / HW))[ch]).astype(np.float16)  # [128, HALVES]
    ab = np.array([[alpha_b.reshape(-1)[0]]], dtype=np.float32)
    ones = np.ones((1, 128), dtype=np.float32)

    # Pack rows as (j*128+p) -> [b=2j both halves | b=2j+1 both halves].
    x16 = x.astype(np.float16)  # [B, C, H, W]
    in_maps = []
    for k in range(NCORES):
        xs = (
            x16[k * BPC : (k + 1) * BPC]
            .reshape(NPAIR, 2, HALVES, 128, HW)
            .transpose(0, 3, 1, 2, 4)
            .reshape(ROWS, 2 * HALVES * HW)
        )
        in_maps.append(
            {"x": np.ascontiguousarray(xs), "wq": wq, "tab": tab, "ab": ab,
             "ones": ones}
        )
    return in_maps


def _unpack_core_out(raw: np.ndarray) -> np.ndarray:
    """Device [ROWS, 2*HALVES*HW] (pair-packed rows) -> [BPC, C, H, W]."""
    return (
        np.asarray(raw)
        .astype(np.float32)
        .reshape(NPAIR, 128, 2, HALVES, HW)
        .transpose(0, 2, 3, 1, 4)
        .reshape(BPC, C, H, W)
    )


def _run(inputs: dict, trace: bool = False, trace_cores=None):
    nc = _get_module()
    in_maps = _prep_in_maps(inputs)
    res = run_bass_kernel_spmd(
        nc, in_maps, core_ids=list(range(NCORES)), trace=trace,
        trace_cores=trace_cores,
    )
    outs = [_unpack_core_out(r["out"]) for r in res.results]
    full = np.concatenate(outs, axis=0)
    return full, res


def kernel(**inputs) -> np.ndarray:
    out, _ = _run(inputs, trace=False)
    return out


# revision 10
# speedup vs baseline: 1.0784x; 1.0784x over previous
"""Trainium2 Bass kernel for nn_CalibratedNorm.

The reference module collapses algebraically to a per-(sample, channel)
affine:

    out[b,c,h,w] = x[b,c,h,w] * A[b,c] + S[b,c]

where, with gs/gsh the folded global-BN scale/shift and ms/msh the folded
mean-of-group-BNs scale/shift (all tiny [C] host math):

    alpha[b] = sigmoid( sum_c (alpha_w[c]/HW) * sum_hw x[b,c,:,:] + alpha_b )
    A[b,c]   = gs[c]  + alpha[b] * (ms[c]  - gs[c])
    S[b,c]   = gsh[c] + alpha[b] * (msh[c] - gsh[c])

Strategy: data-parallel over batch, 4 samples per core on 8 cores. The
kernel is DMA-ring bound (~360-410 GB/s of SBUF-side bytes per core at
the 12544B-per-partition-row packet size), so:

  * x travels the wire as fp16 both ways (host casts): 6.4 MB in +
    6.4 MB out per core vs 25.7 MB round trip in fp32. fp16 costs
    ~6e-4 relative error on this N(0,1) input - far inside the 2e-2
    gate. Bigger rows (25088B) measured no faster; smaller (6272B)
    measured ~344.
  * The host packs each sample as rows (b*128+p) holding both channel
    halves contiguously, one 1.6MB DMA per sample each way.
  * All params ride ONE small HWDGE load issued before the bulk loads
    (lands ~8.5us, before the first alpha needs them). The previous
    SWDGE param path delivered them at 17-21us and stalled the first
    affines for ~2-6us of ring time.
  * The alpha dot runs on the otherwise-idle PE as 14 accumulating
    matmuls per sample (lhsT = 2^16-prescaled weight column, rhs =
    448-pixel chunks of x), then a 448-elem DVE reduce + ACT sigmoid +
    PE ones-broadcast. DVE's 1x-rate full reduce_sum (3.4us per
    half-sample) was the original critical path - the PE dot removes
    it.
  * All bulk affines run on DVE (fp16 tensor_scalar, ~1.1us per
    half-sample), so each sample's store chases its affine closely.
"""

import sys

import numpy as np

for _p in ("/opt/trn_rl_repo",):
    if _p not in sys.path:
        sys.path.insert(0, _p)

import concourse.bacc as bacc
import concourse.bass as bass
import concourse.tile as tile
from concourse import mybir
from concourse.bass_utils import run_bass_kernel_spmd
from concourse.tile import add_dep_helper

EPS = 1e-5
B, C, H, W, G = 32, 256, 56, 56, 32
HW = H * W  # 3136
NCORES = 8
BPC = B // NCORES  # samples per core: 4
HALVES = C // 128  # channel partition-tiles per sample: 2
ROWS = BPC * 128  # 512 rows of the per-core [ROWS, HALVES*HW] x shard
NCHUNK = 7  # alpha-dot rhs chunks per half
CHUNK = HW // NCHUNK  # 448 pixels per chunk (PSUM-bank sized)
WSCALE = 65536.0  # alpha-weight prescale to keep fp16 normal
NPCOL = 11  # params columns: tab(8) + wq32(2) + ab(1)
F32 = mybir.dt.float32
F16 = mybir.dt.float16


def build_module() -> bass.Bass:
    # Bacc (not raw Bass): its compile() pass splits multi-sem waits into
    # EventSemaphore instructions — TRN2 allows at most 1 wait per
    # compute instruction and walrus codegen hard-errors otherwise.
    nc = bacc.Bacc("TRN2")

    x_in = nc.dram_tensor("x", [ROWS, HALVES * HW], F16, kind="ExternalInput")
    par_in = nc.dram_tensor("par", [128, NPCOL], F32, kind="ExternalInput")
    y_out = nc.dram_tensor("out", [ROWS, HALVES * HW], F16, kind="ExternalOutput")

    with tile.TileContext(nc) as tc:
        with (
            tc.tile_pool(name="xp", bufs=BPC) as xp,
            tc.tile_pool(name="cs", bufs=1) as cs,
            tc.tile_pool(name="wk", bufs=2) as wk,
            tc.tile_pool(name="ps", bufs=BPC, space="PSUM") as ps,
            tc.tile_pool(name="pb", bufs=2, space="PSUM") as pb,
        ):
            # One tiny param load, first on the HWDGE ring (~50ns of ring
            # time, lands ~8.5us): tab cols 0-7, wq-fp32 cols 8-9, ab 10.
            par = cs.tile([128, NPCOL], F32)
            nc.sync.dma_start(out=par, in_=par_in[:, :])
            wq = cs.tile([128, HALVES], F16)
            nc.vector.tensor_copy(out=wq, in_=par[:, 8:10])
            ab = par[0:1, 10:11]
            ones_row = cs.tile([1, 128], F32)
            nc.vector.memset(ones_row, 1.0)

            # row r = b*128 + p holds sample b, partition p, both halves:
            # channel c = h*128 + p, pixels contiguous per half.
            xv = x_in[:, :].rearrange("(b p) (h w) -> b p h w", p=128, h=HALVES)
            yv = y_out[:, :].rearrange("(b p) (h w) -> b p h w", p=128, h=HALVES)

            loads = []
            stores = []
            for b in range(BPC):
                xt = xp.tile([128, HALVES, HW], F16, name=f"xt{b}", tag="xt")
                # One 1.6MB DMA per sample: 12544B per partition row.
                loads.append(nc.sync.dma_start(out=xt[:, :, :], in_=xv[b]))

                # alpha dot on PE: zp[0,n] accumulates
                #   sum_h sum_k sum_p wq[p,h] * x[p,h,448k+n]
                zp = ps.tile([1, CHUNK], F32, name=f"zp{b}", tag="zp")
                for h in range(HALVES):
                    for k in range(NCHUNK):
                        nc.tensor.matmul(
                            zp[:, :],
                            lhsT=wq[:, h : h + 1],
                            rhs=xt[:, h, k * CHUNK : (k + 1) * CHUNK],
                            start=(h == 0 and k == 0),
                            stop=(h == HALVES - 1 and k == NCHUNK - 1),
                        )
                # z = sum_n zp[0,n]; alpha = sigmoid(z/WSCALE + alpha_b)
                zs = wk.tile([1, 1], F32, name=f"zs{b}", tag="zs")
                nc.vector.reduce_sum(
                    out=zs, in_=zp[:, :], axis=mybir.AxisListType.X
                )
                al = wk.tile([1, 1], F32, name=f"al{b}", tag="al")
                nc.scalar.activation(
                    out=al, in_=zs[:, :],
                    func=mybir.ActivationFunctionType.Sigmoid,
                    bias=ab, scale=float(1.0 / WSCALE),
                )
                # broadcast alpha to all partitions, move to SBUF
                bc = pb.tile([128, 1], F32, name=f"bc{b}", tag="bc")
                nc.tensor.matmul(
                    bc[:, :], lhsT=ones_row[:, :], rhs=al[:, :],
                    start=True, stop=True,
                )
                ac = wk.tile([128, 1], F32, name=f"ac{b}", tag="ac")
                nc.vector.tensor_copy(out=ac, in_=bc[:, :])

                # A = gs + alpha*dms ; S = gsh + alpha*dmsh   [128, 2]
                A = wk.tile([128, HALVES], F32, name=f"A{b}", tag="A")
                Sh = wk.tile([128, HALVES], F32, name=f"S{b}", tag="S")
                nc.vector.tensor_scalar_mul(out=A, in0=par[:, 2:4], scalar1=ac)
                nc.vector.tensor_add(out=A, in0=A[:, :], in1=par[:, 0:2])
                nc.vector.tensor_scalar_mul(out=Sh, in0=par[:, 6:8], scalar1=ac)
                nc.vector.tensor_add(out=Sh, in0=Sh[:, :], in1=par[:, 4:6])

                # Fused affine on DVE (fp16 tensor_scalar, fp32 scalars);
                # single 1.6MB store per sample.
                for h in range(HALVES):
                    nc.vector.tensor_scalar(
                        out=xt[:, h, :], in0=xt[:, h, :],
                        scalar1=A[:, h : h + 1], scalar2=Sh[:, h : h + 1],
                        op0=mybir.AluOpType.mult, op1=mybir.AluOpType.add,
                    )
                stores.append(nc.sync.dma_start(out=yv[b], in_=xt[:, :, :]))

            # Keep every load ahead of every store in the HWDGE ring:
            # ordering-only edges (no sems) from each store to the last
            # load. Without this the scheduler interleaves stores before
            # the last load, which delays the last sample's alpha chain.
            for st in stores:
                add_dep_helper(
                    st.ins, loads[-1].ins, sync=False,
                    reason="loads drain before stores on SP ring",
                )

    nc.compile()
    return nc


_NC_CACHE: list = []


def _get_module() -> bass.Bass:
    if not _NC_CACHE:
        _NC_CACHE.append(build_module())
    return _NC_CACHE[0]


def _prep_in_maps(inputs: dict) -> list[dict]:
    x = np.asarray(inputs["x"], dtype=np.float32)
    alpha_w = np.asarray(inputs["alpha_w"], dtype=np.float32)
    alpha_b = np.asarray(inputs["alpha_b"], dtype=np.float32)
    g_w = np.asarray(inputs["g_w"], dtype=np.float32)
    g_b = np.asarray(inputs["g_b"], dtype=np.float32)
    g_rm = np.asarray(inputs["g_rm"], dtype=np.float32)
    g_rv = np.asarray(inputs["g_rv"], dtype=np.float32)
    grp_w = np.asarray(inputs["grp_w"], dtype=np.float32)
    grp_b = np.asarray(inputs["grp_b"], dtype=np.float32)
    grp_rm = np.asarray(inputs["grp_rm"], dtype=np.float32)
    grp_rv = np.asarray(inputs["grp_rv"], dtype=np.float32)

    gs = g_w / np.sqrt(g_rv + EPS)
    gsh = g_b - g_rm * gs
    sg = grp_w / np.sqrt(grp_rv + EPS)  # [G, C]
    ms = sg.mean(axis=0)
    msh = (grp_b - grp_rm * sg).mean(axis=0)
    dms = ms - gs
    dmsh = msh - gsh

    ch = (np.arange(HALVES)[None, :] * 128 + np.arange(128)[:, None])  # [128, HALVES]
    par = np.zeros((128, NPCOL), dtype=np.float32)
    par[:, 0:2] = gs[ch]
    par[:, 2:4] = dms[ch]
    par[:, 4:6] = gsh[ch]
    par[:, 6:8] = dmsh[ch]
    par[:, 8:10] = (alpha_w * (WSCALE / HW))[ch]
    par[0, 10] = alpha_b.reshape(-1)[0]

    # Pack rows as (b*128+p) -> [h=0 pixels | h=1 pixels], fp16.
    x16 = x.astype(np.float16)  # [B, C, H, W]
    in_maps = []
    for k in range(NCORES):
        xs = (
            x16[k * BPC : (k + 1) * BPC]
            .reshape(BPC, HALVES, 128, HW)
            .transpose(0, 2, 1, 3)
            .reshape(ROWS, HALVES * HW)
        )
        in_maps.append({"x": np.ascontiguousarray(xs), "par": par})
    return in_maps


def _unpack_core_out(raw: np.ndarray) -> np.ndarray:
    """Device [ROWS, HALVES*HW] (packed rows) -> [BPC, C, H, W] fp32."""
    return (
        np.asarray(raw)
        .astype(np.float32)
        .reshape(BPC, 128, HALVES, HW)
        .transpose(0, 2, 1, 3)
        .reshape(BPC, C, H, W)
    )


def _run(inputs: dict, trace: bool = False, trace_cores=None):
    nc = _get_module()
    in_maps = _prep_in_maps(inputs)
    res = run_bass_kernel_spmd(
        nc, in_maps, core_ids=list(range(NCORES)), trace=trace,
        trace_cores=trace_cores,
    )
    outs = [_unpack_core_out(r["out"]) for r in res.results]
    full = np.concatenate(outs, axis=0)
    return full, res


def kernel(**inputs) -> np.ndarray:
    out, _ = _run(inputs, trace=False)
    return out
